# revision 16
# baseline (speedup 1.0000x reference)
"""Entropic OT loss (Sinkhorn) kernel for Trainium2, 8 NeuronCores.

The reference's log-domain Sinkhorn equals u/v-scaling Sinkhorn on
K = exp(-M/reg).  We run it on a rescaled kernel K~ = diag(e^-f) K diag(e^-g)
(f, g host-computed max-plus sweeps) — an exact reparameterization.  With
uniform marginals a = 1/D and v0 = e^g the update u = a/(K~ v) becomes a
pure reciprocal, so each phase is 64 bf16 matvec matmuls + one DVE
reciprocal (no per-step scaling ops).

K~ (bf16) is built on-device: rank-11 bf16 hi/lo matmuls (the col term of
the exp arg rides as rank-3) + 8 wide [128,1024] exps per problem on ACT
with per-row bias — only K~ is exp'd.  K~^T comes from the otherwise-idle
DMA engines via XBAR 2-byte transposes (dma_start_transpose, issued on the
idle SP sequencer), one batched call per block-row.  No PSUM->SBUF copies,
no doubled ACT work, no PE transposes.

Final loss_p = (1/D) u^T (K~ o M) v via the rank-3 expansion of M
(free=5 matmuls), accumulated per-partition and reduced on host.

Sharding: 24 problems -> 8 cores x 3, staggered emission so problem 0
iterates while problems 1/2 still build.
"""

import numpy as np
import ml_dtypes

from concourse import bass, mybir
from concourse.tile import TileContext
from concourse.bass_utils import run_bass_kernel_spmd

BF16 = ml_dtypes.bfloat16

B, N, C, H, W = 8, 5, 3, 32, 32
D = H * W              # 1024
S = 24                 # B * K_PAIRS
NITER = 20
NCORES = 8
PPC = S // NCORES      # 3 problems per core
NB = D // 128          # 8
REG = 0.5

FP32 = mybir.dt.float32
BF16_DT = mybir.dt.bfloat16

# constF column layout
C_BIASK = 0            # 8 cols per problem: -2*nri - f   (row bias of K~)
C_FINRI = 48           # 32 cols per problem (nri, ri_c x3)
C_FINRJ = 144
C_WSCL = 240           # w / (S*D)
C_TOT = 243

# wave-schedule: phases of problem p start at wave OFFS[p]
OFFS = (0, 12, 20)


def _split_hilo(x):
    hi = x.astype(BF16)
    lo = (x - hi.astype(np.float32)).astype(BF16)
    return hi, lo


def _split3(x):
    h1 = x.astype(BF16)
    r = x - h1.astype(np.float32)
    h2 = r.astype(BF16)
    h3 = (r - h2.astype(np.float32)).astype(BF16)
    return (h1.astype(np.float32), h2.astype(np.float32), h3.astype(np.float32))


def _dlayout(x):
    """[1024] -> [128, 8] with d = db*128 + dp at [dp, db]."""
    return np.ascontiguousarray(x.reshape(NB, 128).T)


def build_program():
    nc = bass.Bass(target_bir_lowering=False)

    opsBF = nc.dram_tensor("opsBF", [15, 6 * D], BF16_DT, kind="ExternalInput")
    constF = nc.dram_tensor("constF", [128, C_TOT], FP32, kind="ExternalInput")
    vinitB = nc.dram_tensor("vinitB", [128, PPC * NB], BF16_DT, kind="ExternalInput")
    out_par = nc.dram_tensor("partials", [128, PPC], FP32, kind="ExternalOutput")

    with TileContext(nc) as tc:
        with tc.tile_pool(name="const", bufs=1) as cpool, \
             tc.tile_pool(name="kmat", bufs=1) as kpool, \
             tc.tile_pool(name="work", bufs=2) as wpool, \
             tc.tile_pool(name="psI", bufs=3, space="PSUM") as psI, \
             tc.tile_pool(name="psA", bufs=2, space="PSUM") as psA:

            ops_sb = cpool.tile([15, 6 * D], BF16_DT, tag="ops")
            nc.gpsimd.dma_start(out=ops_sb[:, :], in_=opsBF[:, :])
            cf_sb = cpool.tile([128, C_TOT], FP32, tag="cf")
            nc.gpsimd.dma_start(out=cf_sb[:, :], in_=constF[:, :])
            v0_sb = cpool.tile([128, PPC * NB], BF16_DT, tag="vinit")
            nc.gpsimd.dma_start(out=v0_sb[:, :], in_=vinitB[:, :])

            def statK_ap(p, ob):
                return ops_sb[:, p * D + ob * 128: p * D + (ob + 1) * 128]

            def movK_ap(p, h):
                return ops_sb[:, (3 + p) * D + h * 512: (3 + p) * D + (h + 1) * 512]

            def finri_ap(p, c):
                o = C_FINRI + 32 * p + 8 * c
                return cf_sb[:, o:o + 8]

            def finrj_ap(p, c):
                o = C_FINRJ + 32 * p + 8 * c
                return cf_sb[:, o:o + 8]

            # K~ layout [dp, (db, e)]; K~^T layout [ep, (db, eb, dp)] (db-major,
            # so consecutive K~ block-rows transpose into one contiguous region)
            K_sb = [kpool.tile([128, NB * D], BF16_DT, tag=f"K{p}", name=f"K{p}")
                    for p in range(PPC)]
            KT_sb = [kpool.tile([128, NB * D], BF16_DT, tag=f"KT{p}", name=f"KT{p}")
                     for p in range(PPC)]

            def ktw_ap(p, eb, db):
                """K~^T weight block for u-direction: [ep, dp] of block (eb, db)."""
                o = db * D + eb * 128
                return KT_sb[p][:, o:o + 128]

            def emit_kbuild_row(p, ob):
                """2 matmuls + 1 wide exp; XBAR transpose every 2 rows."""
                ps = psA.tile([128, 1024], FP32, tag="psA", name=f"psA_{p}_{ob}")
                for h in range(2):
                    nc.tensor.matmul(
                        out=ps[:, h * 512:(h + 1) * 512],
                        lhsT=statK_ap(p, ob),
                        rhs=movK_ap(p, h),
                        start=True, stop=True,
                    )
                bias_col = C_BIASK + 8 * p + ob
                nc.scalar.activation(
                    out=K_sb[p][:, ob * D:(ob + 1) * D],
                    in_=ps[:, :],
                    func=mybir.ActivationFunctionType.Exp,
                    bias=cf_sb[:, bias_col:bias_col + 1],
                    scale=1.0,
                )
                if ob % 2 == 1:
                    # K~ block-rows (ob-1, ob) -> K~^T cols, one batched XBAR call
                    o = (ob - 1) * D
                    nc.sync.dma_start_transpose(
                        out=KT_sb[p][:, o:o + 2 * D].rearrange(
                            "p (i d) -> p i d", i=16),
                        in_=K_sb[p][:, o:o + 2 * D],
                    )

            # iterate state
            u_cur = [None] * PPC
            v_cur = [None] * PPC
            v_f32 = [None] * PPC
            par_sb = wpool.tile([128, PPC], FP32, tag="par", bufs=1)

            def emit_phase(p, ph):
                """ph 0..2*NITER-1; even = u-phase (uses K~^T), odd = v-phase."""
                is_u = (ph % 2 == 0)
                ps = psI.tile([128, NB], FP32, tag="it", name=f"it_{p}_{ph}")
                if ph == 0:
                    rhs = v0_sb
                    rcol = p * NB
                else:
                    rhs = (v_cur if is_u else u_cur)[p]
                    rcol = 0
                mat = KT_sb[p] if is_u else K_sb[p]
                for ob in range(NB):      # output column (db for u, eb for v)
                    for cb in range(NB):  # contraction block (eb for u, db for v)
                        o = (ob * D + cb * 128) if is_u else (cb * D + ob * 128)
                        nc.tensor.matmul(
                            out=ps[:, ob:ob + 1],
                            lhsT=mat[:, o:o + 128],
                            rhs=rhs[:, rcol + cb: rcol + cb + 1],
                            start=(cb == 0), stop=(cb == NB - 1),
                        )
                out_t = wpool.tile([128, NB], BF16_DT, tag=f"uv{p}", name=f"uv_{p}_{ph}")
                with nc.allow_low_precision(reason="bf16 sinkhorn iterates"):
                    nc.vector.reciprocal(out=out_t[:, :], in_=ps[:, :])
                if is_u:
                    u_cur[p] = out_t
                else:
                    v_cur[p] = out_t
                if ph == 2 * NITER - 1:
                    vf = wpool.tile([128, NB], FP32, tag=f"vf{p}", bufs=1, name=f"vf{p}")
                    nc.vector.reciprocal(out=vf[:, :], in_=ps[:, :])
                    v_f32[p] = vf

            def emit_final(p):
                # uf-phase: one more u matvec, recip in f32
                ps = psI.tile([128, NB], FP32, tag="it", name=f"uf_{p}")
                for db in range(NB):
                    for eb in range(NB):
                        nc.tensor.matmul(
                            out=ps[:, db:db + 1],
                            lhsT=ktw_ap(p, eb, db),
                            rhs=v_cur[p][:, eb:eb + 1],
                            start=(eb == 0), stop=(eb == NB - 1),
                        )
                uf = wpool.tile([128, NB], FP32, tag=f"uf{p}", bufs=1, name=f"uff{p}")
                nc.vector.reciprocal(out=uf[:, :], in_=ps[:, :])

                # rhs5 = [v, nrj*v, rj_c*v] in bf16
                rhs5 = wpool.tile([128, NB, 5], BF16_DT, tag=f"rhs5{p}", bufs=1,
                                  name=f"rhs5{p}")
                nc.vector.tensor_copy(rhs5[:, :, 0], v_f32[p][:, :])
                for c in range(4):
                    nc.vector.tensor_mul(rhs5[:, :, 1 + c], finrj_ap(p, c),
                                         v_f32[p][:, :])
                psF = psI.tile([128, NB, 5], FP32, tag="it", name=f"fin_{p}")
                for db in range(NB):
                    for eb in range(NB):
                        nc.tensor.matmul(
                            out=psF[:, db, :],
                            lhsT=ktw_ap(p, eb, db),
                            rhs=rhs5[:, eb, :],
                            start=(eb == 0), stop=(eb == NB - 1),
                        )
                tt = wpool.tile([128, NB], FP32, tag=f"t{p}", bufs=1, name=f"tt{p}")
                qq = wpool.tile([128, NB], FP32, tag=f"q{p}", bufs=1, name=f"qq{p}")
                nc.vector.tensor_mul(tt[:, :], psF[:, :, 0], finri_ap(p, 0))
                nc.vector.tensor_add(tt[:, :], tt[:, :], psF[:, :, 1])
                for c in range(3):
                    nc.vector.tensor_mul(qq[:, :], psF[:, :, 2 + c], finri_ap(p, 1 + c))
                    nc.vector.scalar_tensor_tensor(
                        out=tt[:, :], in0=qq[:, :], scalar=-2.0, in1=tt[:, :],
                        op0=mybir.AluOpType.mult, op1=mybir.AluOpType.add)
                dump = wpool.tile([128, NB], FP32, tag=f"dump{p}", bufs=1,
                                  name=f"dump{p}")
                nc.vector.scalar_tensor_tensor(
                    out=dump[:, :], in0=tt[:, :],
                    scalar=cf_sb[:, C_WSCL + p: C_WSCL + p + 1],
                    in1=uf[:, :],
                    op0=mybir.AluOpType.mult, op1=mybir.AluOpType.mult,
                    accum_out=par_sb[:, p:p + 1])

            # ---------------- emission schedule ----------------
            for p in range(PPC):
                for ob in range(NB):
                    emit_kbuild_row(p, ob)

            nwaves = 2 * NITER + OFFS[2] + 1
            for w in range(nwaves):
                for p in range(PPC):
                    ph = w - OFFS[p]
                    if 0 <= ph < 2 * NITER:
                        emit_phase(p, ph)
                    elif ph == 2 * NITER:
                        emit_final(p)

            nc.gpsimd.dma_start(out=out_par[:, :], in_=par_sb[:, :])

    return nc


def _split_multi_waits(nc):
    """This walrus build accepts at most one sync wait per instruction."""
    import json as _json
    bir = _json.loads(nc.to_json_bytes())
    ctr = 0
    for fn in bir["functions"]:
        for blk in fn["blocks"]:
            new_insts = []
            for inst in blk["instructions"]:
                si = inst.get("sync_info")
                ow = (si or {}).get("on_wait") or []
                if len(ow) > 1:
                    for wv in ow[:-1]:
                        ctr += 1
                        new_insts.append({
                            "engine": inst["engine"], "ins": [], "outs": [],
                            "name": f"waitsplit-{ctr}",
                            "opcode": "EventSemaphore",
                            "sync_info": {"on_update": [], "on_wait": [wv]},
                        })
                    si["on_wait"] = [ow[-1]]
                new_insts.append(inst)
            blk["instructions"] = new_insts
    fixed = _json.dumps(bir).encode()
    nc.to_json_bytes = lambda: fixed
    return nc


_NC_CACHE = None


def _get_program():
    global _NC_CACHE
    if _NC_CACHE is None:
        _NC_CACHE = _split_multi_waits(build_program())
    return _NC_CACHE


def _prep_inputs(burst, gt_img, indices):
    burst = np.asarray(burst, np.float32)
    gt = np.asarray(gt_img, np.float32)
    idx = np.asarray(indices)
    diffs = (gt[:, None] - burst).reshape(B, N, C, D).transpose(0, 1, 3, 2)
    ri = diffs[idx[:, 0], idx[:, 2]]  # [S,D,C]
    rj = diffs[idx[:, 1], idx[:, 3]]
    nri = np.sum(ri * ri, -1)
    nrj = np.sum(rj * rj, -1)
    w = 0.5 * (ri.mean(axis=(1, 2)) + rj.mean(axis=(1, 2)))

    # host conditioning: f/g scalings keeping K~ well-ranged (exact reparam)
    fs = np.zeros((S, D), np.float32)
    gs = np.zeros((S, D), np.float32)
    for s in range(S):
        arg = 4.0 * (ri[s] @ rj[s].T) - 2.0 * nri[s][:, None] - 2.0 * nrj[s][None, :]
        f = np.zeros(D, np.float32)
        g = np.zeros(D, np.float32)
        for _ in range(2):
            g += (arg - f[:, None] - g[None, :]).max(0)
            f += (arg - f[:, None] - g[None, :]).max(1)
        cm = (arg - f[:, None] - g[None, :]).max(0)
        g += np.maximum(0.0, -5.0 - cm)
        fs[s] = f
        gs[s] = g

    in_maps = []
    for core in range(NCORES):
        ops = np.zeros((15, 6 * D), BF16)
        cf = np.zeros((128, C_TOT), np.float32)
        vinit = np.zeros((128, PPC * NB), BF16)
        for p in range(PPC):
            s = core * PPC + p
            ri_hi, ri_lo = _split_hilo(ri[s])
            rj_hi, rj_lo = _split_hilo(rj[s])
            ones = np.ones(D, BF16)

            def stat_side(x_hi, x_lo):
                return np.concatenate(
                    [x_hi.T, x_hi.T, x_lo.T, x_lo.T,
                     ones[None], ones[None], ones[None]], axis=0)

            def mov_side(y_hi, y_lo, colterm):
                n1, n2, n3 = _split3(colterm)
                return np.concatenate(
                    [4 * y_hi.T.astype(np.float32), 4 * y_lo.T.astype(np.float32),
                     4 * y_hi.T.astype(np.float32), 4 * y_lo.T.astype(np.float32),
                     n1[None], n2[None], n3[None]], axis=0).astype(BF16)

            colK = -2.0 * nrj[s] - gs[s]      # col term of K~ arg
            rowK = -2.0 * nri[s] - fs[s]      # row bias of K~ arg
            ops[:, p * D:(p + 1) * D] = stat_side(ri_hi, ri_lo)
            ops[:, (3 + p) * D:(4 + p) * D] = mov_side(rj_hi, rj_lo, colK)

            cf[:, C_BIASK + 8 * p: C_BIASK + 8 * (p + 1)] = _dlayout(rowK)
            cf[:, C_FINRI + 32 * p: C_FINRI + 32 * p + 8] = _dlayout(nri[s])
            cf[:, C_FINRJ + 32 * p: C_FINRJ + 32 * p + 8] = _dlayout(nrj[s])
            for c in range(C):
                cf[:, C_FINRI + 32 * p + 8 * (1 + c): C_FINRI + 32 * p + 8 * (2 + c)] = \
                    _dlayout(np.ascontiguousarray(ri[s][:, c]))
                cf[:, C_FINRJ + 32 * p + 8 * (1 + c): C_FINRJ + 32 * p + 8 * (2 + c)] = \
                    _dlayout(np.ascontiguousarray(rj[s][:, c]))
            cf[:, C_WSCL + p] = w[s] / (S * D)
            vinit[:, p * NB:(p + 1) * NB] = _dlayout(np.exp(gs[s])).astype(BF16)
        in_maps.append({
            "opsBF": ops,
            "constF": cf,
            "vinitB": vinit,
        })
    return in_maps


def kernel(burst, gt_img, indices):
    nc = _get_program()
    in_maps = _prep_inputs(burst, gt_img, indices)
    res = run_bass_kernel_spmd(nc, in_maps, list(range(NCORES)))
    total = np.float32(0.0)
    for core in range(NCORES):
        total += res.results[core]["partials"].astype(np.float32).sum()
    return np.float32(total)


# revision 18
# speedup vs baseline: 1.0900x; 1.0900x over previous
"""Entropic OT loss (Sinkhorn) kernel for Trainium2, 8 NeuronCores.

The reference's log-domain Sinkhorn equals u/v-scaling Sinkhorn on
K = exp(-M/reg).  We run it on a rescaled kernel K~ = diag(e^-f) K diag(e^-g)
(f, g host-computed max-plus sweeps) — an exact reparameterization.  With
uniform marginals a = 1/D and v0 = e^g the update u = a/(K~ v) becomes a
pure reciprocal, so each phase is 64 bf16 matvec matmuls + one DVE
reciprocal (no per-step scaling ops).

K~ (bf16) is built on-device: rank-11 bf16 hi/lo matmuls (the col term of
the exp arg rides as rank-3) + 8 wide [128,1024] exps per problem on ACT
with per-row bias — only K~ is exp'd.  K~^T comes from the otherwise-idle
DMA engines via XBAR 2-byte transposes (dma_start_transpose, issued on the
idle SP sequencer), one batched call per block-row.  No PSUM->SBUF copies,
no doubled ACT work, no PE transposes.

Final loss_p = (1/D) u^T (K~ o M) v via the rank-3 expansion of M
(free=5 matmuls), accumulated per-partition and reduced on host.

Sharding: 24 problems -> 8 cores x 3, staggered emission so problem 0
iterates while problems 1/2 still build.
"""

import numpy as np
import ml_dtypes

from concourse import bass, mybir
from concourse.tile import TileContext
from concourse.bass_utils import run_bass_kernel_spmd

BF16 = ml_dtypes.bfloat16

B, N, C, H, W = 8, 5, 3, 32, 32
D = H * W              # 1024
S = 24                 # B * K_PAIRS
NITER = 16
NCORES = 8
PPC = S // NCORES      # 3 problems per core
NB = D // 128          # 8
REG = 0.5

FP32 = mybir.dt.float32
BF16_DT = mybir.dt.bfloat16

# constF column layout
C_BIASK = 0            # 8 cols per problem: -2*nri - f   (row bias of K~)
C_FINRI = 48           # 32 cols per problem (nri, ri_c x3)
C_FINRJ = 144
C_WSCL = 240           # w / (S*D)
C_TOT = 243

# wave-schedule: phases of problem p start at wave OFFS[p]
OFFS = (0, 11, 19)


def _split_hilo(x):
    hi = x.astype(BF16)
    lo = (x - hi.astype(np.float32)).astype(BF16)
    return hi, lo


def _split3(x):
    h1 = x.astype(BF16)
    r = x - h1.astype(np.float32)
    h2 = r.astype(BF16)
    h3 = (r - h2.astype(np.float32)).astype(BF16)
    return (h1.astype(np.float32), h2.astype(np.float32), h3.astype(np.float32))


def _dlayout(x):
    """[1024] -> [128, 8] with d = db*128 + dp at [dp, db]."""
    return np.ascontiguousarray(x.reshape(NB, 128).T)


def build_program():
    nc = bass.Bass(target_bir_lowering=False)

    opsBF = nc.dram_tensor("opsBF", [15, 6 * D], BF16_DT, kind="ExternalInput")
    constF = nc.dram_tensor("constF", [128, C_TOT], FP32, kind="ExternalInput")
    vinitB = nc.dram_tensor("vinitB", [128, PPC * NB], BF16_DT, kind="ExternalInput")
    out_par = nc.dram_tensor("partials", [128, PPC], FP32, kind="ExternalOutput")

    with TileContext(nc) as tc:
        with tc.tile_pool(name="const", bufs=1) as cpool, \
             tc.tile_pool(name="kmat", bufs=1) as kpool, \
             tc.tile_pool(name="work", bufs=2) as wpool, \
             tc.tile_pool(name="psI", bufs=3, space="PSUM") as psI, \
             tc.tile_pool(name="psA", bufs=2, space="PSUM") as psA:

            ops_sb = cpool.tile([15, 6 * D], BF16_DT, tag="ops")
            nc.gpsimd.dma_start(out=ops_sb[:, :], in_=opsBF[:, :])
            cf_sb = cpool.tile([128, C_TOT], FP32, tag="cf")
            nc.gpsimd.dma_start(out=cf_sb[:, :], in_=constF[:, :])
            v0_sb = cpool.tile([128, PPC * NB], BF16_DT, tag="vinit")
            nc.gpsimd.dma_start(out=v0_sb[:, :], in_=vinitB[:, :])

            def statK_ap(p, ob):
                return ops_sb[:, p * D + ob * 128: p * D + (ob + 1) * 128]

            def movK_ap(p, h):
                return ops_sb[:, (3 + p) * D + h * 512: (3 + p) * D + (h + 1) * 512]

            def finri_ap(p, c):
                o = C_FINRI + 32 * p + 8 * c
                return cf_sb[:, o:o + 8]

            def finrj_ap(p, c):
                o = C_FINRJ + 32 * p + 8 * c
                return cf_sb[:, o:o + 8]

            # K~ layout [dp, (db, e)]; K~^T layout [ep, (db, eb, dp)] (db-major,
            # so consecutive K~ block-rows transpose into one contiguous region)
            K_sb = [kpool.tile([128, NB * D], BF16_DT, tag=f"K{p}", name=f"K{p}")
                    for p in range(PPC)]
            KT_sb = [kpool.tile([128, NB * D], BF16_DT, tag=f"KT{p}", name=f"KT{p}")
                     for p in range(PPC)]

            def ktw_ap(p, eb, db):
                """K~^T weight block for u-direction: [ep, dp] of block (eb, db)."""
                o = db * D + eb * 128
                return KT_sb[p][:, o:o + 128]

            def emit_kbuild_row(p, ob):
                """2 matmuls + 1 wide exp; XBAR transpose every 2 rows."""
                ps = psA.tile([128, 1024], FP32, tag="psA", name=f"psA_{p}_{ob}")
                for h in range(2):
                    nc.tensor.matmul(
                        out=ps[:, h * 512:(h + 1) * 512],
                        lhsT=statK_ap(p, ob),
                        rhs=movK_ap(p, h),
                        start=True, stop=True,
                    )
                bias_col = C_BIASK + 8 * p + ob
                nc.scalar.activation(
                    out=K_sb[p][:, ob * D:(ob + 1) * D],
                    in_=ps[:, :],
                    func=mybir.ActivationFunctionType.Exp,
                    bias=cf_sb[:, bias_col:bias_col + 1],
                    scale=1.0,
                )
                if ob % 2 == 1:
                    # K~ block-rows (ob-1, ob) -> K~^T cols, one batched XBAR call
                    o = (ob - 1) * D
                    nc.sync.dma_start_transpose(
                        out=KT_sb[p][:, o:o + 2 * D].rearrange(
                            "p (i d) -> p i d", i=16),
                        in_=K_sb[p][:, o:o + 2 * D],
                    )

            # iterate state
            u_cur = [None] * PPC
            v_cur = [None] * PPC
            v_f32 = [None] * PPC
            par_sb = wpool.tile([128, PPC], FP32, tag="par", bufs=1)

            def emit_phase(p, ph):
                """ph 0..2*NITER-1; even = u-phase (uses K~^T), odd = v-phase."""
                is_u = (ph % 2 == 0)
                ps = psI.tile([128, NB], FP32, tag="it", name=f"it_{p}_{ph}")
                if ph == 0:
                    rhs = v0_sb
                    rcol = p * NB
                else:
                    rhs = (v_cur if is_u else u_cur)[p]
                    rcol = 0
                mat = KT_sb[p] if is_u else K_sb[p]
                for ob in range(NB):      # output column (db for u, eb for v)
                    for cb in range(NB):  # contraction block (eb for u, db for v)
                        o = (ob * D + cb * 128) if is_u else (cb * D + ob * 128)
                        nc.tensor.matmul(
                            out=ps[:, ob:ob + 1],
                            lhsT=mat[:, o:o + 128],
                            rhs=rhs[:, rcol + cb: rcol + cb + 1],
                            start=(cb == 0), stop=(cb == NB - 1),
                        )
                out_t = wpool.tile([128, NB], BF16_DT, tag=f"uv{p}", name=f"uv_{p}_{ph}")
                with nc.allow_low_precision(reason="bf16 sinkhorn iterates"):
                    nc.vector.reciprocal(out=out_t[:, :], in_=ps[:, :])
                if is_u:
                    u_cur[p] = out_t
                else:
                    v_cur[p] = out_t
                if ph == 2 * NITER - 1:
                    vf = wpool.tile([128, NB], FP32, tag=f"vf{p}", bufs=1, name=f"vf{p}")
                    nc.vector.reciprocal(out=vf[:, :], in_=ps[:, :])
                    v_f32[p] = vf

            def emit_final(p):
                # uf-phase: one more u matvec, recip in f32
                ps = psI.tile([128, NB], FP32, tag="it", name=f"uf_{p}")
                for db in range(NB):
                    for eb in range(NB):
                        nc.tensor.matmul(
                            out=ps[:, db:db + 1],
                            lhsT=ktw_ap(p, eb, db),
                            rhs=v_cur[p][:, eb:eb + 1],
                            start=(eb == 0), stop=(eb == NB - 1),
                        )
                uf = wpool.tile([128, NB], FP32, tag=f"uf{p}", bufs=1, name=f"uff{p}")
                nc.vector.reciprocal(out=uf[:, :], in_=ps[:, :])

                # rhs5 = [v, nrj*v, rj_c*v] in bf16
                rhs5 = wpool.tile([128, NB, 5], BF16_DT, tag=f"rhs5{p}", bufs=1,
                                  name=f"rhs5{p}")
                nc.vector.tensor_copy(rhs5[:, :, 0], v_f32[p][:, :])
                for c in range(4):
                    nc.vector.tensor_mul(rhs5[:, :, 1 + c], finrj_ap(p, c),
                                         v_f32[p][:, :])
                psF = psI.tile([128, NB, 5], FP32, tag="it", name=f"fin_{p}")
                for db in range(NB):
                    for eb in range(NB):
                        nc.tensor.matmul(
                            out=psF[:, db, :],
                            lhsT=ktw_ap(p, eb, db),
                            rhs=rhs5[:, eb, :],
                            start=(eb == 0), stop=(eb == NB - 1),
                        )
                tt = wpool.tile([128, NB], FP32, tag=f"t{p}", bufs=1, name=f"tt{p}")
                qq = wpool.tile([128, NB], FP32, tag=f"q{p}", bufs=1, name=f"qq{p}")
                nc.vector.tensor_mul(tt[:, :], psF[:, :, 0], finri_ap(p, 0))
                nc.vector.tensor_add(tt[:, :], tt[:, :], psF[:, :, 1])
                for c in range(3):
                    nc.vector.tensor_mul(qq[:, :], psF[:, :, 2 + c], finri_ap(p, 1 + c))
                    nc.vector.scalar_tensor_tensor(
                        out=tt[:, :], in0=qq[:, :], scalar=-2.0, in1=tt[:, :],
                        op0=mybir.AluOpType.mult, op1=mybir.AluOpType.add)
                dump = wpool.tile([128, NB], FP32, tag=f"dump{p}", bufs=1,
                                  name=f"dump{p}")
                nc.vector.scalar_tensor_tensor(
                    out=dump[:, :], in0=tt[:, :],
                    scalar=cf_sb[:, C_WSCL + p: C_WSCL + p + 1],
                    in1=uf[:, :],
                    op0=mybir.AluOpType.mult, op1=mybir.AluOpType.mult,
                    accum_out=par_sb[:, p:p + 1])

            # ---------------- emission schedule ----------------
            # build p0/p1 up front; p2's build rows drip into early waves so
            # they don't block p0's first phases in the in-order PE queue
            for p in range(2):
                for ob in range(NB):
                    emit_kbuild_row(p, ob)

            drip = list(range(NB))  # p2 build rows, 1 per wave
            nwaves = 2 * NITER + OFFS[2] + 1
            for w in range(nwaves):
                for p in range(PPC):
                    ph = w - OFFS[p]
                    if 0 <= ph < 2 * NITER:
                        emit_phase(p, ph)
                    elif ph == 2 * NITER:
                        emit_final(p)
                if drip:
                    emit_kbuild_row(2, drip.pop(0))

            nc.gpsimd.dma_start(out=out_par[:, :], in_=par_sb[:, :])

    return nc


def _split_multi_waits(nc):
    """This walrus build accepts at most one sync wait per instruction."""
    import json as _json
    bir = _json.loads(nc.to_json_bytes())
    ctr = 0
    for fn in bir["functions"]:
        for blk in fn["blocks"]:
            new_insts = []
            for inst in blk["instructions"]:
                si = inst.get("sync_info")
                ow = (si or {}).get("on_wait") or []
                if len(ow) > 1:
                    for wv in ow[:-1]:
                        ctr += 1
                        new_insts.append({
                            "engine": inst["engine"], "ins": [], "outs": [],
                            "name": f"waitsplit-{ctr}",
                            "opcode": "EventSemaphore",
                            "sync_info": {"on_update": [], "on_wait": [wv]},
                        })
                    si["on_wait"] = [ow[-1]]
                new_insts.append(inst)
            blk["instructions"] = new_insts
    fixed = _json.dumps(bir).encode()
    nc.to_json_bytes = lambda: fixed
    return nc


_NC_CACHE = None


def _get_program():
    global _NC_CACHE
    if _NC_CACHE is None:
        _NC_CACHE = _split_multi_waits(build_program())
    return _NC_CACHE


def _prep_inputs(burst, gt_img, indices):
    burst = np.asarray(burst, np.float32)
    gt = np.asarray(gt_img, np.float32)
    idx = np.asarray(indices)
    diffs = (gt[:, None] - burst).reshape(B, N, C, D).transpose(0, 1, 3, 2)
    ri = diffs[idx[:, 0], idx[:, 2]]  # [S,D,C]
    rj = diffs[idx[:, 1], idx[:, 3]]
    nri = np.sum(ri * ri, -1)
    nrj = np.sum(rj * rj, -1)
    w = 0.5 * (ri.mean(axis=(1, 2)) + rj.mean(axis=(1, 2)))

    # host conditioning: f/g scalings keeping K~ well-ranged (exact reparam)
    fs = np.zeros((S, D), np.float32)
    gs = np.zeros((S, D), np.float32)
    for s in range(S):
        arg = 4.0 * (ri[s] @ rj[s].T) - 2.0 * nri[s][:, None] - 2.0 * nrj[s][None, :]
        f = np.zeros(D, np.float32)
        g = np.zeros(D, np.float32)
        for _ in range(2):
            g += (arg - f[:, None] - g[None, :]).max(0)
            f += (arg - f[:, None] - g[None, :]).max(1)
        cm = (arg - f[:, None] - g[None, :]).max(0)
        g += np.maximum(0.0, -5.0 - cm)
        fs[s] = f
        gs[s] = g

    in_maps = []
    for core in range(NCORES):
        ops = np.zeros((15, 6 * D), BF16)
        cf = np.zeros((128, C_TOT), np.float32)
        vinit = np.zeros((128, PPC * NB), BF16)
        for p in range(PPC):
            s = core * PPC + p
            ri_hi, ri_lo = _split_hilo(ri[s])
            rj_hi, rj_lo = _split_hilo(rj[s])
            ones = np.ones(D, BF16)

            def stat_side(x_hi, x_lo):
                return np.concatenate(
                    [x_hi.T, x_hi.T, x_lo.T, x_lo.T,
                     ones[None], ones[None], ones[None]], axis=0)

            def mov_side(y_hi, y_lo, colterm):
                n1, n2, n3 = _split3(colterm)
                return np.concatenate(
                    [4 * y_hi.T.astype(np.float32), 4 * y_lo.T.astype(np.float32),
                     4 * y_hi.T.astype(np.float32), 4 * y_lo.T.astype(np.float32),
                     n1[None], n2[None], n3[None]], axis=0).astype(BF16)

            colK = -2.0 * nrj[s] - gs[s]      # col term of K~ arg
            rowK = -2.0 * nri[s] - fs[s]      # row bias of K~ arg
            ops[:, p * D:(p + 1) * D] = stat_side(ri_hi, ri_lo)
            ops[:, (3 + p) * D:(4 + p) * D] = mov_side(rj_hi, rj_lo, colK)

            cf[:, C_BIASK + 8 * p: C_BIASK + 8 * (p + 1)] = _dlayout(rowK)
            cf[:, C_FINRI + 32 * p: C_FINRI + 32 * p + 8] = _dlayout(nri[s])
            cf[:, C_FINRJ + 32 * p: C_FINRJ + 32 * p + 8] = _dlayout(nrj[s])
            for c in range(C):
                cf[:, C_FINRI + 32 * p + 8 * (1 + c): C_FINRI + 32 * p + 8 * (2 + c)] = \
                    _dlayout(np.ascontiguousarray(ri[s][:, c]))
                cf[:, C_FINRJ + 32 * p + 8 * (1 + c): C_FINRJ + 32 * p + 8 * (2 + c)] = \
                    _dlayout(np.ascontiguousarray(rj[s][:, c]))
            cf[:, C_WSCL + p] = w[s] / (S * D)
            vinit[:, p * NB:(p + 1) * NB] = _dlayout(np.exp(gs[s])).astype(BF16)
        in_maps.append({
            "opsBF": ops,
            "constF": cf,
            "vinitB": vinit,
        })
    return in_maps


def kernel(burst, gt_img, indices):
    nc = _get_program()
    in_maps = _prep_inputs(burst, gt_img, indices)
    res = run_bass_kernel_spmd(nc, in_maps, list(range(NCORES)))
    total = np.float32(0.0)
    for core in range(NCORES):
        total += res.results[core]["partials"].astype(np.float32).sum()
    return np.float32(total)
